# revision 49
# baseline (speedup 1.0000x reference)
"""Trainium2 Bass kernel for nn_BiLSTM_7928509628689.

Masked bidirectional LSTM over N=2048 ragged sequences (T=64, D=512, H=256),
returning concat of final fwd/bwd hidden states [N, 2H].

Strategy (8 NeuronCores, data-parallel over N, 256 seqs/core):
  * Sequences are globally sorted by length (desc) and dealt round-robin to
    cores, so all cores carry a near-identical length profile. All
    sequences are right-aligned in time (they END at the last step), so at
    step s only the V_s longest sequences are active. V_s is baked into
    the program: every matmul / ACT / DVE op at step s is trimmed to V_s
    columns. Mean length is ~T/2, so this halves the PE columns.
  * All state kept TRANSPOSED: hT/cT [H, Ns] folded into persistent
    [128, 2*Ns] tiles updated in place (never-yet-active columns stay 0).
  * Per step and direction, gates^T [4H, V_s] are built in PSUM (bank
    order g,i,f,o), one accumulation group per bank:
       4 matmuls  W_ih^T chunks @ x_s chunks    (bf16 input projection)
       h-projection: 2 bf16 matmuls per region (small steps), or ONE fp8
         e4m3 DoubleRow matmul per region (steps with V_s > 128) -- the
         DVE h-update writes the h state directly as fp8; the e4m3 noise
         on the recurrence measures ~1.4e-2 rel, inside the 2e-2 budget
       1 matmul   [b; mask_coef] @ [ones; maskinv_s]  (bias + pad forcing)
    K=128 for every bf16 matmul so LDWEIGHTS stays FWL-pipelined.
  * PSUM rotation for lookahead: V_s <= 64 packs all 8 gate regions into
    ONE bank rotated over 4 banks (x-proj runs up to 4 steps ahead of the
    recurrent chain); V_s <= 128 uses the 2-bank parity scheme; larger
    steps fill all 4 banks per direction, ordered i-bank first so the next
    step's x-proj never head-of-line blocks on a slow bank.
  * Pad forcing: columns included before their sequence's first step get
    -40 on the i/f/o pre-activations, so their state is forced to ~0 until
    the sequence starts; the final state at the last step is exactly the
    masked-LSTM output for both directions (bwd consumes the time-reversed
    sequence).
  * ACT: tanh over the g bank, per-bank sigmoids (i / f / o split so each
    bank frees as soon as its group stops), one tanh(c); DVE does the
    elementwise updates on exact active ranges.

kernel(**inputs) takes the FULL unsharded inputs and returns [2048, 512] f32.
"""
import numpy as np

import concourse.tile as tile
from concourse import bacc, mybir
from concourse.bass_utils import run_bass_kernel_spmd
import bass_rust

F32 = mybir.dt.float32
BF16 = mybir.dt.bfloat16
AF = mybir.ActivationFunctionType
OP = mybir.AluOpType

N, T, D, H = 2048, 64, 512, 256
NCORES = 8
NS = N // NCORES           # 256 sequences per core
FH = 4 * H                 # 1024 gate rows
KD = D // 128              # 4 x-projection K chunks
KH = H // 128              # 2 h-projection K chunks
FORCE = -40.0              # gate penalty for pad steps
MB = 8                     # mask rhs block (steps per mask DMA)
VSPLIT = 1 << 30           # fp8 DoubleRow x-proj disabled: e4m3 noise busts 2e-2
PACKLIM = 128              # steps with v <= PACKLIM use the packed 2-bank layout
BIAS_DVE_V = 192           # DVE bias path only when v > this (else PE has slack)
DIRS = ("f", "b")
FP8 = mybir.dt.float8e4
PM = mybir.MatmulPerfMode
# PSUM bank order within the [128, 4*512] gates tile; sigmoid spans i,f,o
BANK_MS = ((4, 5), (0, 1), (2, 3), (6, 7))   # g, i, f, o
BANK_OF = [b * 512 for b in range(4)]

_NC_CACHE = {}


def _inst(r):
    return getattr(r, "ins", r)


def _build(t_steps, V):
    import contextlib

    nc = bacc.Bacc("TRN2", target_bir_lowering=False, debug=False)

    # first step using the fp8 DoubleRow x-projection (V is nondecreasing)
    s0 = next((s for s in range(t_steps) if int(V[s]) > VSPLIT), t_steps)
    t8 = t_steps - s0

    x_dram = {}
    x8_dram, wih8_d = {}, {}
    wih_d, whh_d, bm_d, out_d = {}, {}, {}, {}
    for d in DIRS:
        # x stored [t, 128, KD, NS]: (p, k) <-> input dim  dd = KD*p + k
        x_dram[d] = nc.dram_tensor(
            f"x{d}", [t_steps, 128, KD, NS], BF16, kind="ExternalInput"
        ).ap()
        wih_d[d] = nc.dram_tensor(
            f"wih{d}", [128, KD, FH], BF16, kind="ExternalInput"
        ).ap()
        if t8 > 0:
            # fp8 x for DR steps: [t8, 128, pr, ko, NS]; dd = pr*256+ko*128+p
            x8_dram[d] = nc.dram_tensor(
                f"x8{d}", [t8, 128, 2, 2, NS], FP8, kind="ExternalInput"
            ).ap()
            wih8_d[d] = nc.dram_tensor(
                f"wih8{d}", [128, 2, 2, FH], FP8, kind="ExternalInput"
            ).ap()
        whh_d[d] = nc.dram_tensor(
            f"whh{d}", [128, KH, FH], BF16, kind="ExternalInput"
        ).ap()
        bm_d[d] = nc.dram_tensor(f"bm{d}", [128, FH], BF16, kind="ExternalInput").ap()
        out_d[d] = nc.dram_tensor(
            f"hT{d}", [128, KH * NS], F32, kind="ExternalOutput"
        ).ap()
    mask_d = nc.dram_tensor(
        "maskrhs", [128, t_steps * NS], BF16, kind="ExternalInput"
    ).ap()
    # per-chunk gate bias, replicated over columns; chunk axis in physical
    # order [g4,g5,i0,i1,f2,f3,o6,o7]; used by the DVE bias path (late steps)
    any_np = any(int(V[s]) > PACKLIM for s in range(t_steps))
    first_np = next(
        (s for s in range(t_steps) if int(V[s]) > PACKLIM), t_steps
    )
    h8_from = max(first_np - 1, 0)   # steps >= this write the fp8 h state
    biasc_d, whh8_d = {}, {}
    if any_np:
        for d in DIRS:
            # per-chunk gate bias columns (order g4,g5,i0,i1,f2,f3,o6,o7)
            # consumed by the ACT bias operand on wide steps
            biasc_d[d] = nc.dram_tensor(
                f"biasc{d}", [128, 8], F32, kind="ExternalInput"
            ).ap()
            # fp8 recurrent weights for the DoubleRow h-projection
            whh8_d[d] = nc.dram_tensor(
                f"whh8{d}", [128, KH, FH], FP8, kind="ExternalInput"
            ).ap()

    with tile.TileContext(nc) as tc:
        with contextlib.ExitStack() as ctx:
            wpool = ctx.enter_context(tc.tile_pool(name="w", bufs=1))
            xpool = ctx.enter_context(tc.tile_pool(name="x", bufs=6))
            mpool = ctx.enter_context(tc.tile_pool(name="mask", bufs=2))
            spool = ctx.enter_context(tc.tile_pool(name="state", bufs=1))
            opool = ctx.enter_context(tc.tile_pool(name="outs", bufs=1))
            apool = ctx.enter_context(tc.tile_pool(name="acts", bufs=2))
            pspool = ctx.enter_context(tc.tile_pool(name="ps", bufs=1, space="PSUM"))

            wih_t, whh_t, bm_t, wih8_t = {}, {}, {}, {}
            for d in DIRS:
                wih_t[d] = wpool.tile([128, KD, FH], BF16, tag=f"wih_{d}", name=f"wih_{d}")
                for k in range(KD):
                    nc.gpsimd.dma_start(wih_t[d][:, k], wih_d[d][:, k])
                whh_t[d] = wpool.tile([128, KH, FH], BF16, tag=f"whh_{d}", name=f"whh_{d}")
                nc.gpsimd.dma_start(whh_t[d][:], whh_d[d][:])
                bm_t[d] = wpool.tile([128, FH], BF16, tag=f"bm_{d}", name=f"bm_{d}")
                nc.gpsimd.dma_start(bm_t[d][:], bm_d[d][:])
                if t8 > 0:
                    wih8_t[d] = wpool.tile(
                        [128, 2, 2, FH], FP8, tag=f"wih8_{d}", name=f"wih8_{d}"
                    )
                    nc.gpsimd.dma_start(wih8_t[d][:], wih8_d[d][:])
            biasc_t, whh8_t, h8_t = {}, {}, {}
            if any_np:
                for d in DIRS:
                    biasc_t[d] = wpool.tile(
                        [128, 8], F32, tag=f"biasc_{d}", name=f"biasc_{d}"
                    )
                    nc.gpsimd.dma_start(biasc_t[d][:], biasc_d[d][:])
                    whh8_t[d] = wpool.tile(
                        [128, KH, FH], FP8, tag=f"whh8_{d}", name=f"whh8_{d}"
                    )
                    nc.gpsimd.dma_start(whh8_t[d][:], whh8_d[d][:])
                    h8_t[d] = spool.tile(
                        [128, KH * NS], FP8, tag=f"h8_{d}", name=f"h8_{d}"
                    )
                    nc.vector.memset(h8_t[d][:], 0.0)

            # persistent state tiles, updated in place; inactive columns
            # stay zero from this init
            h_t, c_t = {}, {}
            for d in DIRS:
                h_t[d] = spool.tile([128, KH * NS], BF16, tag=f"h_{d}", name=f"h_{d}")
                nc.vector.memset(h_t[d][:], 0.0)
                c_t[d] = spool.tile([128, KH * NS], F32, tag=f"c_{d}", name=f"c_{d}")
                nc.vector.memset(c_t[d][:], 0.0)

            # persistent per-direction gates tiles (4 PSUM banks each);
            # bank-granular dep tracking lets a step's matmuls overlap the
            # previous step's ACT reads when they touch different banks
            ps_t = {}
            for d in DIRS:
                ps_t[d] = pspool.tile(
                    [128, 4 * 512], F32, tag=f"ps_{d}", name=f"ps_{d}"
                )

            # PE warm-up burst: dense dummy matmuls during the initial
            # weight/x DMA window so HAM reaches full clock before step 0;
            # parked on the bwd dir's last rotation bank (used latest)
            wrm = wpool.tile([128, 512], BF16, tag="warm", name="warm")
            nc.vector.memset(wrm[:], 0.0)
            NWARM = 36
            for i in range(NWARM):
                nc.tensor.matmul(
                    ps_t["b"][:, 1536 : 1536 + 128], wrm[:, 0:128], wrm[:, 0:128],
                    start=(i == 0), stop=(i == NWARM - 1),
                )

            def mk_layout(s, v):
                """Per-step PSUM layout + regime flags (same for both dirs)."""
                quad = v <= 64
                packed = not quad and v <= PACKLIM
                L = {"s": s, "v": v, "quad": quad, "packed": packed,
                     "dve_bias": not quad and not packed and v > BIAS_DVE_V,
                     "last": s == t_steps - 1}
                if quad:
                    # all 8 half-ranges in ONE bank; rotate over 4 banks so
                    # x-proj can run up to 4 steps ahead of the chain
                    base = 512 * (s % 4)
                    L["banks"] = [
                        [(base + j * 64, m)
                         for j, m in enumerate((4, 5, 0, 1, 2, 3, 6, 7))],
                    ]
                    L["bankset"] = {s % 4}
                    L["sl"] = (base, base + 128, base + 384, base + 512)
                elif packed:
                    # 2 banks; alternate bank set by step parity
                    base = 1024 * (s % 2)
                    L["banks"] = [
                        [(base + j * 128, m) for j, m in enumerate((4, 5, 0, 1))],
                        [(base + 512 + j * 128, m)
                         for j, m in enumerate((2, 3, 6, 7))],
                    ]
                    L["bankset"] = {2 * (s % 2), 2 * (s % 2) + 1}
                    L["sl"] = (base, base + 256, base + 768, base + 1024)
                else:
                    # i bank first: it frees earliest, so the next step's
                    # x-proj never head-of-line blocks on a slow bank
                    L["banks"] = [
                        [(b * 512 + half * NS, BANK_MS[b][half])
                         for half in range(2)]
                        for b in (1, 0, 2, 3)
                    ]
                    L["bankset"] = {0, 1, 2, 3}
                return L

            def emit_pass1(L, d, xt, mrhs, fp8_step):
                ps = ps_t[d]
                v = L["v"]
                bias_mm = L["quad"] or L["packed"] or not L["dve_bias"]
                for regs in L["banks"]:
                    start_mm = None
                    for idx, (off, m) in enumerate(regs):
                        o_ap = ps[:, off : off + v]
                        msl = slice(m * 128, (m + 1) * 128)
                        if fp8_step:
                            r = nc.tensor.matmul(
                                o_ap, wih8_t[d][:, 0, :, msl], xt[:, 0, :, :v],
                                start=(idx == 0), stop=False,
                                perf_mode=PM.DoubleRow,
                            )
                        else:
                            r = nc.tensor.matmul(
                                o_ap, wih_t[d][:, 0, msl], xt[:, 0, :v],
                                start=(idx == 0), stop=False,
                            )
                        if idx == 0:
                            start_mm = _inst(r)
                        else:
                            # later regions rely on the bank-wide has_written
                            # clear done by the start matmul
                            bass_rust.add_dep_helper(
                                _inst(r), start_mm, sync=False,
                                reason="psum bank group order",
                            )
                        if fp8_step:
                            nc.tensor.matmul(
                                o_ap, wih8_t[d][:, 1, :, msl], xt[:, 1, :, :v],
                                start=False, stop=False,
                                perf_mode=PM.DoubleRow,
                            )
                        else:
                            for k in range(1, KD):
                                nc.tensor.matmul(
                                    o_ap, wih_t[d][:, k, msl], xt[:, k, :v],
                                    start=False, stop=False,
                                )
                        if bias_mm or m in (0, 1):
                            # bias row + -40 pad forcing; when the DVE bias
                            # path is on, only the i chunks keep it (for the
                            # pad forcing)
                            nc.tensor.matmul(
                                o_ap, bm_t[d][:, msl], mrhs,
                                start=False, stop=False,
                            )

            def emit_rest(L, d):
                """h-projection + activations + state update for step L."""
                ps = ps_t[d]
                v = L["v"]

                fp8_h = not L["quad"] and not L["packed"]
                if fp8_h:
                    h8v = h8_t[d].rearrange("p (q n) -> p q n", q=2)[:, :, :v]
                for regs in L["banks"]:
                    for idx, (off, m) in enumerate(regs):
                        o_ap = ps[:, off : off + v]
                        msl = slice(m * 128, (m + 1) * 128)
                        if fp8_h:
                            # one DoubleRow matmul contracts both h chunks
                            nc.tensor.matmul(
                                o_ap, whh8_t[d][:, :, msl], h8v,
                                start=False, stop=(idx == len(regs) - 1),
                                perf_mode=PM.DoubleRow,
                            )
                        else:
                            for kk in range(KH):
                                nc.tensor.matmul(
                                    o_ap,
                                    whh_t[d][:, kk, msl],
                                    h_t[d][:, kk * NS : kk * NS + v],
                                    start=False,
                                    stop=(idx == len(regs) - 1 and kk == KH - 1),
                                )

                def v2(ap, q):
                    return ap.rearrange("p (q n) -> p q n", q=q)[:, :, :v]

                tg = apool.tile([128, 512], F32, tag=f"tg_{d}", name=f"tg_{d}")
                t1 = apool.tile([128, 512], F32, tag=f"t1_{d}", name=f"t1_{d}")
                tcn = apool.tile([128, 512], F32, tag=f"tc_{d}", name=f"tc_{d}")
                cc = c_t[d]
                if L["quad"] or L["packed"]:
                    lo, a, b_, hi = L["sl"]
                    tg_sl, si_sl, so_sl = ps[:, lo:a], ps[:, a:b_], ps[:, b_:hi]
                    nc.scalar.activation(v2(tg[:], 2), v2(tg_sl, 2), AF.Tanh)
                    if L["quad"]:
                        # quad band is ACT-op-bound: i,f,o are contiguous in
                        # the single bank, one 6-range sigmoid does all three
                        sio = apool.tile(
                            [128, 3 * 512], F32, tag=f"sio_{d}", name=f"sio_{d}"
                        )
                        nc.scalar.activation(
                            v2(sio[:], 6), v2(ps[:, a:hi], 6), AF.Sigmoid
                        )
                        sig_i, sig_f = sio[:, 0:512], sio[:, 512:1024]
                        sig_o = sio[:, 1024:1536]
                    else:
                        si = apool.tile(
                            [128, 2 * 512], F32, tag=f"si_{d}", name=f"si_{d}"
                        )
                        nc.scalar.activation(v2(si[:], 4), v2(si_sl, 4), AF.Sigmoid)
                        so = apool.tile([128, 512], F32, tag=f"so_{d}", name=f"so_{d}")
                        nc.scalar.activation(v2(so[:], 2), v2(so_sl, 2), AF.Sigmoid)
                        sig_i, sig_f = si[:, 0:512], si[:, 512:1024]
                        sig_o = so[:]
                else:
                    tg_sl, si_sl = ps[:, 0:512], ps[:, 512:1024]
                    sf_sl, so2_sl = ps[:, 1024:1536], ps[:, 1536:2048]
                    si = apool.tile([128, 512], F32, tag=f"si2_{d}", name=f"si2_{d}")
                    nc.scalar.activation(v2(si[:], 2), v2(si_sl, 2), AF.Sigmoid)
                    sf = apool.tile([128, 512], F32, tag=f"sf_{d}", name=f"sf_{d}")
                    so = apool.tile([128, 512], F32, tag=f"so2_{d}", name=f"so2_{d}")
                    if L["dve_bias"]:
                        # per-chunk ACTs with the bias operand: the add rides
                        # the activation, keeping DVE off the gate chain
                        bc = biasc_t[d]
                        for half in range(2):
                            o2 = half * 256
                            nc.scalar.activation(
                                tg[:, o2 : o2 + v], tg_sl[:, o2 : o2 + v],
                                AF.Tanh, bias=bc[:, half : half + 1],
                            )
                            nc.scalar.activation(
                                sf[:, o2 : o2 + v], sf_sl[:, o2 : o2 + v],
                                AF.Sigmoid, bias=bc[:, 4 + half : 5 + half],
                            )
                            nc.scalar.activation(
                                so[:, o2 : o2 + v], so2_sl[:, o2 : o2 + v],
                                AF.Sigmoid, bias=bc[:, 6 + half : 7 + half],
                            )
                    else:
                        nc.scalar.activation(v2(tg[:], 2), v2(tg_sl, 2), AF.Tanh)
                        nc.scalar.activation(v2(sf[:], 2), v2(sf_sl, 2), AF.Sigmoid)
                        nc.scalar.activation(v2(so[:], 2), v2(so2_sl, 2), AF.Sigmoid)
                    sig_i, sig_f = si[:], sf[:]
                    sig_o = so[:]

                nc.vector.tensor_tensor(
                    v2(t1[:], 2), v2(sig_i, 2), v2(tg[:], 2), OP.mult
                )
                nc.vector.tensor_tensor(
                    v2(cc[:], 2), v2(sig_f, 2), v2(cc[:], 2), OP.mult
                )
                nc.vector.tensor_tensor(
                    v2(cc[:], 2), v2(cc[:], 2), v2(t1[:], 2), OP.add
                )
                nc.scalar.activation(v2(tcn[:], 2), v2(cc[:], 2), AF.Tanh)
                if L["last"]:
                    hf = opool.tile([128, 512], F32, tag=f"hout_{d}", name=f"hout_{d}")
                    nc.vector.tensor_tensor(hf[:], sig_o, tcn[:], OP.mult)
                    nc.sync.dma_start(out_d[d][:], hf[:])
                elif any_np and L["s"] >= h8_from:
                    # next step's h-projection runs in fp8 DoubleRow
                    nc.vector.tensor_tensor(
                        v2(h8_t[d][:], 2), v2(sig_o, 2), v2(tcn[:], 2), OP.mult
                    )
                else:
                    nc.vector.tensor_tensor(
                        v2(h_t[d][:], 2), v2(sig_o, 2), v2(tcn[:], 2), OP.mult
                    )

            # software pipeline: emit step s's x-projection before step
            # s-1's recurrence/chain whenever their PSUM banks are disjoint
            # (rotation bands), so the in-order PE queue never head-of-line
            # blocks lookahead work behind a stalled h-projection
            mtile = None
            pend = None
            for s in range(t_steps):
                v = int(V[s])
                fp8_step = s >= s0
                xts = {}
                for d in DIRS:
                    if fp8_step:
                        xts[d] = xpool.tile(
                            [128, 2, 2, NS], FP8, tag=f"x8_{d}", name=f"x8_{d}"
                        )
                        nc.sync.dma_start(
                            xts[d][:, :, :, :v], x8_dram[d][s - s0][:, :, :, :v]
                        )
                    else:
                        xts[d] = xpool.tile(
                            [128, KD, NS], BF16, tag=f"x_{d}", name=f"x_{d}"
                        )
                        nc.sync.dma_start(xts[d][:, :, :v], x_dram[d][s][:, :, :v])
                if s % MB == 0:
                    mw = min(MB, t_steps - s) * NS
                    mtile = mpool.tile([128, MB * NS], BF16, tag="m", name="mtile")
                    nc.sync.dma_start(
                        mtile[:, :mw], mask_d[:, s * NS : s * NS + mw]
                    )
                mrhs = mtile[:, (s % MB) * NS : (s % MB) * NS + v]

                L = mk_layout(s, v)
                for d in DIRS:
                    emit_pass1(L, d, xts[d], mrhs, fp8_step)
                    emit_rest(L, d)

    nc.compile()
    return nc


def _get_nc(t_steps, V):
    key = (t_steps, tuple(V))
    if key not in _NC_CACHE:
        _NC_CACHE[key] = _build(t_steps, V)
    return _NC_CACHE[key]


def _prep_weights(W_ih, W_hh, b):
    """lhsT layouts for one direction."""
    import ml_dtypes

    wdt = ml_dtypes.bfloat16
    f8 = ml_dtypes.float8_e4m3
    wih = np.ascontiguousarray(
        W_ih.T.reshape(128, KD, FH).astype(wdt)
    )  # (p, k) <-> dd = KD*p + k
    # fp8 DoubleRow layout: [p, pr, ko, FH] with dd = pr*256 + ko*128 + p
    wih8 = np.ascontiguousarray(
        W_ih.T.reshape(2, 2, 128, FH).transpose(2, 0, 1, 3).astype(f8)
    )
    whh = np.ascontiguousarray(
        W_hh.T.reshape(KH, 128, FH).transpose(1, 0, 2).astype(wdt)
    )  # (p, kk) <-> hrow = 128*kk + p
    whh8 = np.ascontiguousarray(
        W_hh.T.reshape(KH, 128, FH).transpose(1, 0, 2).astype(f8)
    )
    coef = np.zeros(FH, np.float32)
    coef[: 2 * H] = FORCE       # i, f gates
    coef[3 * H :] = FORCE       # o gate
    bm = np.zeros((128, FH), np.float32)
    bm[0] = b.astype(np.float32)
    bm[1] = coef
    bm = np.ascontiguousarray(bm.astype(wdt))
    # ACT bias columns: chunk order [g4,g5,i0,i1,f2,f3,o6,o7]; i slots zero
    # (i gets bias+mask via the bm matmul)
    border = (4, 5, 0, 1, 2, 3, 6, 7)
    b8 = np.stack([b[c * 128 : (c + 1) * 128] for c in border], axis=1)  # [128, 8]
    b8[:, 2:4] = 0.0
    biasc = np.ascontiguousarray(b8.astype(np.float32))
    return wih, whh, bm, wih8, biasc, whh8


def _prep_core(seqs_c, lens_c, t_steps, s0):
    """Per-core device arrays. seqs_c [NS, T, D], lens_c [NS] (sorted desc)."""
    import ml_dtypes

    bf16 = ml_dtypes.bfloat16
    f8 = ml_dtypes.float8_e4m3
    ns = seqs_c.shape[0]
    shift = t_steps - lens_c  # pad steps per sequence
    src_t = np.arange(t_steps)[None, :] - shift[:, None]      # [NS, t]
    valid = src_t >= 0
    gat = seqs_c[np.arange(ns)[:, None], np.clip(src_t, 0, T - 1)]
    xf = np.where(valid[..., None], gat, np.float32(0.0))     # right-aligned
    xb = seqs_c[:, t_steps - 1 :: -1, :]                      # time-reversed

    def to_dev(x_ntd):
        # [NS, t, D] -> bf16 [t, 128, KD, NS] (dd = KD*p + k) for steps < s0,
        # fp8 [t8, 128, 2, 2, NS] (dd = pr*256 + ko*128 + p) for steps >= s0
        xt = x_ntd.transpose(1, 2, 0)                          # [t, D, NS]
        xbf = np.ascontiguousarray(
            xt.astype(bf16).reshape(t_steps, 128, KD, ns))
        t8 = t_steps - s0
        if t8 > 0:
            x8 = np.ascontiguousarray(
                xt[s0:].reshape(t8, 2, 2, 128, ns)
                .transpose(0, 3, 1, 2, 4).astype(f8))
        else:
            x8 = None
        return xbf, x8

    maskinv = (np.arange(t_steps)[:, None] < shift[None, :]).astype(np.float32)
    maskrhs = np.zeros((128, t_steps * ns), np.float32)
    maskrhs[0] = 1.0
    maskrhs[1] = maskinv.reshape(t_steps * ns)
    maskrhs = np.ascontiguousarray(maskrhs.astype(bf16))
    xfb, xf8 = to_dev(xf)
    xbb, xb8 = to_dev(xb)
    return {"xf": xfb, "xb": xbb, "xf8": xf8, "xb8": xb8, "maskrhs": maskrhs}


def _unfold(hT):
    """[128, KH*NS] device tile -> [NS, H] h matrix."""
    h_rows = np.concatenate([hT[:, i * NS : (i + 1) * NS] for i in range(KH)], axis=0)
    return h_rows.T  # [NS, H]


def _run(inputs, trace=False, t_cap=None, **spmd_kwargs):
    import ml_dtypes

    all_embs = np.asarray(inputs["all_embs"], dtype=np.float32)
    lengths = np.asarray(inputs["lengths"]).astype(np.int64)
    starts = np.asarray(inputs["starts"]).astype(np.int64)

    if np.array_equal(starts, np.arange(N, dtype=np.int64) * T):
        seqs = all_embs.reshape(N, T, D)
    else:
        seqs = all_embs[starts[:, None] + np.arange(T)[None, :]]

    # global sort by length desc, deal round-robin to cores
    order = np.argsort(-lengths, kind="stable")
    t_steps = int(lengths.max())
    if t_cap is not None:
        t_steps = min(t_steps, t_cap)
    core_idx = [order[c::NCORES] for c in range(NCORES)]  # [NCORES][NS]

    # baked active widths: V_s = max over cores of #{len >= t_steps - s}
    Ls = np.stack([np.minimum(lengths[ci], t_steps) for ci in core_idx])  # [NC, NS]
    thr = t_steps - np.arange(t_steps)  # [t]
    V = (Ls[:, None, :] >= thr[None, :, None]).sum(-1).max(0)  # [t]
    V = np.maximum(V, 1)

    w = {}
    for d, (wi, wh, bb) in {
        "f": (inputs["W_ih_f"], inputs["W_hh_f"], inputs["b_f"]),
        "b": (inputs["W_ih_b"], inputs["W_hh_b"], inputs["b_b"]),
    }.items():
        w[d] = _prep_weights(
            np.asarray(wi, np.float32), np.asarray(wh, np.float32),
            np.asarray(bb, np.float32),
        )

    s0 = next((s for s in range(t_steps) if int(V[s]) > VSPLIT), t_steps)
    in_maps = []
    for ci in range(NCORES):
        idx = core_idx[ci]
        m = _prep_core(seqs[idx], np.minimum(lengths[idx], t_steps), t_steps, s0)
        im = {
            "xf": m["xf"], "xb": m["xb"], "maskrhs": m["maskrhs"],
            "wihf": w["f"][0], "whhf": w["f"][1], "bmf": w["f"][2],
            "wihb": w["b"][0], "whhb": w["b"][1], "bmb": w["b"][2],
        }
        if m["xf8"] is not None:
            im.update(
                {
                    "x8f": m["xf8"], "x8b": m["xb8"],
                    "wih8f": w["f"][3], "wih8b": w["b"][3],
                }
            )
        if int(V.max()) > PACKLIM:
            im.update(
                {
                    "biascf": w["f"][4], "biascb": w["b"][4],
                    "whh8f": w["f"][5], "whh8b": w["b"][5],
                }
            )
        in_maps.append(im)

    nc = _get_nc(t_steps, V)
    res = None
    for attempt in range(3):
        try:
            res = run_bass_kernel_spmd(
                nc, in_maps, core_ids=list(range(NCORES)), trace=trace,
                **spmd_kwargs
            )
            break
        except Exception:
            # rare transient NRT_EXEC_UNIT_UNRECOVERABLE right after a
            # fresh NEFF load; a plain re-execute has always recovered
            if attempt == 2:
                raise
            import time as _time

            _time.sleep(2.0)

    out = np.empty((N, 2 * H), np.float32)
    for ci in range(NCORES):
        out[core_idx[ci], :H] = _unfold(res.results[ci]["hTf"])
        out[core_idx[ci], H:] = _unfold(res.results[ci]["hTb"])
    return out, res


def kernel(**inputs) -> np.ndarray:
    out, _ = _run(inputs)
    return out



# revision 50
# speedup vs baseline: 1.0218x; 1.0218x over previous
"""Trainium2 Bass kernel for nn_BiLSTM_7928509628689.

Masked bidirectional LSTM over N=2048 ragged sequences (T=64, D=512, H=256),
returning concat of final fwd/bwd hidden states [N, 2H].

Strategy (8 NeuronCores, data-parallel over N, 256 seqs/core):
  * Sequences are globally sorted by length (desc) and dealt round-robin to
    cores, so all cores carry a near-identical length profile. All
    sequences are right-aligned in time (they END at the last step), so at
    step s only the V_s longest sequences are active. V_s is baked into
    the program: every matmul / ACT / DVE op at step s is trimmed to V_s
    columns. Mean length is ~T/2, so this halves the PE columns.
  * All state kept TRANSPOSED: hT/cT [H, Ns] folded into persistent
    [128, 2*Ns] tiles updated in place (never-yet-active columns stay 0).
  * Per step and direction, gates^T [4H, V_s] are built in PSUM (bank
    order g,i,f,o), one accumulation group per bank:
       4 matmuls  W_ih^T chunks @ x_s chunks    (bf16 input projection)
       h-projection: 2 bf16 matmuls per region (small steps), or ONE fp8
         e4m3 DoubleRow matmul per region (steps with V_s > 128) -- the
         DVE h-update writes the h state directly as fp8; the e4m3 noise
         on the recurrence measures ~1.4e-2 rel, inside the 2e-2 budget
       1 matmul   [b; mask_coef] @ [ones; maskinv_s]  (bias + pad forcing)
    K=128 for every bf16 matmul so LDWEIGHTS stays FWL-pipelined.
  * PSUM rotation for lookahead: V_s <= 64 packs all 8 gate regions into
    ONE bank rotated over 4 banks (x-proj runs up to 4 steps ahead of the
    recurrent chain); V_s <= 128 uses the 2-bank parity scheme; larger
    steps fill all 4 banks per direction, ordered i-bank first so the next
    step's x-proj never head-of-line blocks on a slow bank.
  * Pad forcing: columns included before their sequence's first step get
    -40 on the i/f/o pre-activations, so their state is forced to ~0 until
    the sequence starts; the final state at the last step is exactly the
    masked-LSTM output for both directions (bwd consumes the time-reversed
    sequence).
  * ACT: tanh over the g bank, per-bank sigmoids (i / f / o split so each
    bank frees as soon as its group stops), one tanh(c); DVE does the
    elementwise updates on exact active ranges.

kernel(**inputs) takes the FULL unsharded inputs and returns [2048, 512] f32.
"""
import numpy as np

import concourse.tile as tile
from concourse import bacc, mybir
from concourse.bass_utils import run_bass_kernel_spmd
import bass_rust

F32 = mybir.dt.float32
BF16 = mybir.dt.bfloat16
AF = mybir.ActivationFunctionType
OP = mybir.AluOpType

N, T, D, H = 2048, 64, 512, 256
NCORES = 8
NS = N // NCORES           # 256 sequences per core
FH = 4 * H                 # 1024 gate rows
KD = D // 128              # 4 x-projection K chunks
KH = H // 128              # 2 h-projection K chunks
FORCE = -40.0              # gate penalty for pad steps
MB = 8                     # mask rhs block (steps per mask DMA)
VSPLIT = 1 << 30           # fp8 DoubleRow x-proj disabled: e4m3 noise busts 2e-2
PACKLIM = 128              # steps with v <= PACKLIM use the packed 2-bank layout
BIAS_DVE_V = 192           # DVE bias path only when v > this (else PE has slack)
DIRS = ("f", "b")
FP8 = mybir.dt.float8e4
PM = mybir.MatmulPerfMode
# PSUM bank order within the [128, 4*512] gates tile; sigmoid spans i,f,o
BANK_MS = ((4, 5), (0, 1), (2, 3), (6, 7))   # g, i, f, o
BANK_OF = [b * 512 for b in range(4)]

_NC_CACHE = {}


def _inst(r):
    return getattr(r, "ins", r)


def _build(t_steps, V):
    import contextlib

    nc = bacc.Bacc("TRN2", target_bir_lowering=False, debug=False)

    # first step using the fp8 DoubleRow x-projection (V is nondecreasing)
    s0 = next((s for s in range(t_steps) if int(V[s]) > VSPLIT), t_steps)
    t8 = t_steps - s0

    x_dram = {}
    x8_dram, wih8_d = {}, {}
    wih_d, whh_d, bm_d, out_d = {}, {}, {}, {}
    for d in DIRS:
        # x stored [t, 128, KD, NS]: (p, k) <-> input dim  dd = KD*p + k
        x_dram[d] = nc.dram_tensor(
            f"x{d}", [t_steps, 128, KD, NS], BF16, kind="ExternalInput"
        ).ap()
        wih_d[d] = nc.dram_tensor(
            f"wih{d}", [128, KD, FH], BF16, kind="ExternalInput"
        ).ap()
        if t8 > 0:
            # fp8 x for DR steps: [t8, 128, pr, ko, NS]; dd = pr*256+ko*128+p
            x8_dram[d] = nc.dram_tensor(
                f"x8{d}", [t8, 128, 2, 2, NS], FP8, kind="ExternalInput"
            ).ap()
            wih8_d[d] = nc.dram_tensor(
                f"wih8{d}", [128, 2, 2, FH], FP8, kind="ExternalInput"
            ).ap()
        whh_d[d] = nc.dram_tensor(
            f"whh{d}", [128, KH, FH], BF16, kind="ExternalInput"
        ).ap()
        bm_d[d] = nc.dram_tensor(f"bm{d}", [128, FH], BF16, kind="ExternalInput").ap()
        out_d[d] = nc.dram_tensor(
            f"hT{d}", [128, KH * NS], F32, kind="ExternalOutput"
        ).ap()
    mask_d = nc.dram_tensor(
        "maskrhs", [128, t_steps * NS], BF16, kind="ExternalInput"
    ).ap()
    # per-chunk gate bias, replicated over columns; chunk axis in physical
    # order [g4,g5,i0,i1,f2,f3,o6,o7]; used by the DVE bias path (late steps)
    any_np = any(int(V[s]) > PACKLIM for s in range(t_steps))
    first_np = next(
        (s for s in range(t_steps) if int(V[s]) > PACKLIM), t_steps
    )
    h8_from = max(first_np - 1, 0)   # steps >= this write the fp8 h state
    biasc_d, whh8_d = {}, {}
    if any_np:
        for d in DIRS:
            # per-chunk gate bias columns (order g4,g5,i0,i1,f2,f3,o6,o7)
            # consumed by the ACT bias operand on wide steps
            biasc_d[d] = nc.dram_tensor(
                f"biasc{d}", [128, 8], F32, kind="ExternalInput"
            ).ap()
            # fp8 recurrent weights for the DoubleRow h-projection
            whh8_d[d] = nc.dram_tensor(
                f"whh8{d}", [128, KH, FH], FP8, kind="ExternalInput"
            ).ap()

    with tile.TileContext(nc) as tc:
        with contextlib.ExitStack() as ctx:
            wpool = ctx.enter_context(tc.tile_pool(name="w", bufs=1))
            xpool = ctx.enter_context(tc.tile_pool(name="x", bufs=6))
            mpool = ctx.enter_context(tc.tile_pool(name="mask", bufs=2))
            spool = ctx.enter_context(tc.tile_pool(name="state", bufs=1))
            opool = ctx.enter_context(tc.tile_pool(name="outs", bufs=1))
            apool = ctx.enter_context(tc.tile_pool(name="acts", bufs=2))
            pspool = ctx.enter_context(tc.tile_pool(name="ps", bufs=1, space="PSUM"))

            wih_t, whh_t, bm_t, wih8_t = {}, {}, {}, {}
            for d in DIRS:
                wih_t[d] = wpool.tile([128, KD, FH], BF16, tag=f"wih_{d}", name=f"wih_{d}")
                for k in range(KD):
                    nc.gpsimd.dma_start(wih_t[d][:, k], wih_d[d][:, k])
                whh_t[d] = wpool.tile([128, KH, FH], BF16, tag=f"whh_{d}", name=f"whh_{d}")
                nc.gpsimd.dma_start(whh_t[d][:], whh_d[d][:])
                bm_t[d] = wpool.tile([128, FH], BF16, tag=f"bm_{d}", name=f"bm_{d}")
                nc.gpsimd.dma_start(bm_t[d][:], bm_d[d][:])
                if t8 > 0:
                    wih8_t[d] = wpool.tile(
                        [128, 2, 2, FH], FP8, tag=f"wih8_{d}", name=f"wih8_{d}"
                    )
                    nc.gpsimd.dma_start(wih8_t[d][:], wih8_d[d][:])
            biasc_t, whh8_t, h8_t = {}, {}, {}
            if any_np:
                for d in DIRS:
                    biasc_t[d] = wpool.tile(
                        [128, 8], F32, tag=f"biasc_{d}", name=f"biasc_{d}"
                    )
                    nc.gpsimd.dma_start(biasc_t[d][:], biasc_d[d][:])
                    whh8_t[d] = wpool.tile(
                        [128, KH, FH], FP8, tag=f"whh8_{d}", name=f"whh8_{d}"
                    )
                    nc.gpsimd.dma_start(whh8_t[d][:], whh8_d[d][:])
                    h8_t[d] = spool.tile(
                        [128, KH * NS], FP8, tag=f"h8_{d}", name=f"h8_{d}"
                    )
                    nc.vector.memset(h8_t[d][:], 0.0)

            # persistent state tiles, updated in place; inactive columns
            # stay zero from this init
            h_t, c_t = {}, {}
            for d in DIRS:
                h_t[d] = spool.tile([128, KH * NS], BF16, tag=f"h_{d}", name=f"h_{d}")
                nc.vector.memset(h_t[d][:], 0.0)
                c_t[d] = spool.tile([128, KH * NS], F32, tag=f"c_{d}", name=f"c_{d}")
                nc.vector.memset(c_t[d][:], 0.0)

            # persistent per-direction gates tiles (4 PSUM banks each);
            # bank-granular dep tracking lets a step's matmuls overlap the
            # previous step's ACT reads when they touch different banks
            ps_t = {}
            for d in DIRS:
                ps_t[d] = pspool.tile(
                    [128, 4 * 512], F32, tag=f"ps_{d}", name=f"ps_{d}"
                )

            # PE warm-up burst: dense dummy matmuls during the initial
            # weight/x DMA window so HAM reaches full clock before step 0;
            # parked on the bwd dir's last rotation bank (used latest)
            wrm = wpool.tile([128, 512], BF16, tag="warm", name="warm")
            nc.vector.memset(wrm[:], 0.0)
            NWARM = 36
            for i in range(NWARM):
                nc.tensor.matmul(
                    ps_t["b"][:, 1536 : 1536 + 128], wrm[:, 0:128], wrm[:, 0:128],
                    start=(i == 0), stop=(i == NWARM - 1),
                )

            def mk_layout(s, v):
                """Per-step PSUM layout + regime flags (same for both dirs)."""
                quad = v <= 64
                packed = not quad and v <= PACKLIM
                L = {"s": s, "v": v, "quad": quad, "packed": packed,
                     "dve_bias": not quad and not packed and v > BIAS_DVE_V,
                     "last": s == t_steps - 1}
                if quad:
                    # all 8 half-ranges in ONE bank; rotate over 4 banks so
                    # x-proj can run up to 4 steps ahead of the chain
                    base = 512 * (s % 4)
                    L["banks"] = [
                        [(base + j * 64, m)
                         for j, m in enumerate((4, 5, 0, 1, 2, 3, 6, 7))],
                    ]
                    L["bankset"] = {s % 4}
                    L["sl"] = (base, base + 128, base + 384, base + 512)
                elif packed:
                    # 2 banks; alternate bank set by step parity
                    base = 1024 * (s % 2)
                    L["banks"] = [
                        [(base + j * 128, m) for j, m in enumerate((4, 5, 0, 1))],
                        [(base + 512 + j * 128, m)
                         for j, m in enumerate((2, 3, 6, 7))],
                    ]
                    L["bankset"] = {2 * (s % 2), 2 * (s % 2) + 1}
                    L["sl"] = (base, base + 256, base + 768, base + 1024)
                else:
                    # i bank first: it frees earliest, so the next step's
                    # x-proj never head-of-line blocks on a slow bank
                    L["banks"] = [
                        [(b * 512 + half * NS, BANK_MS[b][half])
                         for half in range(2)]
                        for b in (1, 0, 2, 3)
                    ]
                    L["bankset"] = {0, 1, 2, 3}
                return L

            def emit_pass1(L, d, xt, mrhs, fp8_step):
                ps = ps_t[d]
                v = L["v"]
                bias_mm = L["quad"] or L["packed"] or not L["dve_bias"]
                for regs in L["banks"]:
                    start_mm = None
                    for idx, (off, m) in enumerate(regs):
                        o_ap = ps[:, off : off + v]
                        msl = slice(m * 128, (m + 1) * 128)
                        if fp8_step:
                            r = nc.tensor.matmul(
                                o_ap, wih8_t[d][:, 0, :, msl], xt[:, 0, :, :v],
                                start=(idx == 0), stop=False,
                                perf_mode=PM.DoubleRow,
                            )
                        else:
                            r = nc.tensor.matmul(
                                o_ap, wih_t[d][:, 0, msl], xt[:, 0, :v],
                                start=(idx == 0), stop=False,
                            )
                        if idx == 0:
                            start_mm = _inst(r)
                        else:
                            # later regions rely on the bank-wide has_written
                            # clear done by the start matmul
                            bass_rust.add_dep_helper(
                                _inst(r), start_mm, sync=False,
                                reason="psum bank group order",
                            )
                        if fp8_step:
                            nc.tensor.matmul(
                                o_ap, wih8_t[d][:, 1, :, msl], xt[:, 1, :, :v],
                                start=False, stop=False,
                                perf_mode=PM.DoubleRow,
                            )
                        else:
                            for k in range(1, KD):
                                nc.tensor.matmul(
                                    o_ap, wih_t[d][:, k, msl], xt[:, k, :v],
                                    start=False, stop=False,
                                )
                        if bias_mm or m in (0, 1):
                            # bias row + -40 pad forcing; when the DVE bias
                            # path is on, only the i chunks keep it (for the
                            # pad forcing)
                            nc.tensor.matmul(
                                o_ap, bm_t[d][:, msl], mrhs,
                                start=False, stop=False,
                            )

            def emit_rest(L, d):
                """h-projection + activations + state update for step L."""
                ps = ps_t[d]
                v = L["v"]

                fp8_h = not L["quad"] and not L["packed"]
                if fp8_h:
                    h8v = h8_t[d].rearrange("p (q n) -> p q n", q=2)[:, :, :v]
                for regs in L["banks"]:
                    for idx, (off, m) in enumerate(regs):
                        o_ap = ps[:, off : off + v]
                        msl = slice(m * 128, (m + 1) * 128)
                        if fp8_h:
                            # one DoubleRow matmul contracts both h chunks
                            nc.tensor.matmul(
                                o_ap, whh8_t[d][:, :, msl], h8v,
                                start=False, stop=(idx == len(regs) - 1),
                                perf_mode=PM.DoubleRow,
                            )
                        else:
                            for kk in range(KH):
                                nc.tensor.matmul(
                                    o_ap,
                                    whh_t[d][:, kk, msl],
                                    h_t[d][:, kk * NS : kk * NS + v],
                                    start=False,
                                    stop=(idx == len(regs) - 1 and kk == KH - 1),
                                )

                def v2(ap, q):
                    return ap.rearrange("p (q n) -> p q n", q=q)[:, :, :v]

                tg = apool.tile([128, 512], F32, tag=f"tg_{d}", name=f"tg_{d}")
                t1 = apool.tile([128, 512], F32, tag=f"t1_{d}", name=f"t1_{d}")
                tcn = apool.tile([128, 512], F32, tag=f"tc_{d}", name=f"tc_{d}")
                cc = c_t[d]
                if L["quad"] or L["packed"]:
                    lo, a, b_, hi = L["sl"]
                    tg_sl, si_sl, so_sl = ps[:, lo:a], ps[:, a:b_], ps[:, b_:hi]
                    nc.scalar.activation(v2(tg[:], 2), v2(tg_sl, 2), AF.Tanh)
                    si = apool.tile([128, 2 * 512], F32, tag=f"si_{d}", name=f"si_{d}")
                    nc.scalar.activation(v2(si[:], 4), v2(si_sl, 4), AF.Sigmoid)
                    so = apool.tile([128, 512], F32, tag=f"so_{d}", name=f"so_{d}")
                    nc.scalar.activation(v2(so[:], 2), v2(so_sl, 2), AF.Sigmoid)
                    sig_i, sig_f = si[:, 0:512], si[:, 512:1024]
                    sig_o = so[:]
                else:
                    tg_sl, si_sl = ps[:, 0:512], ps[:, 512:1024]
                    sf_sl, so2_sl = ps[:, 1024:1536], ps[:, 1536:2048]
                    si = apool.tile([128, 512], F32, tag=f"si2_{d}", name=f"si2_{d}")
                    nc.scalar.activation(v2(si[:], 2), v2(si_sl, 2), AF.Sigmoid)
                    sf = apool.tile([128, 512], F32, tag=f"sf_{d}", name=f"sf_{d}")
                    so = apool.tile([128, 512], F32, tag=f"so2_{d}", name=f"so2_{d}")
                    if L["dve_bias"]:
                        # per-chunk ACTs with the bias operand: the add rides
                        # the activation, keeping DVE off the gate chain
                        bc = biasc_t[d]
                        for half in range(2):
                            o2 = half * 256
                            nc.scalar.activation(
                                tg[:, o2 : o2 + v], tg_sl[:, o2 : o2 + v],
                                AF.Tanh, bias=bc[:, half : half + 1],
                            )
                            nc.scalar.activation(
                                sf[:, o2 : o2 + v], sf_sl[:, o2 : o2 + v],
                                AF.Sigmoid, bias=bc[:, 4 + half : 5 + half],
                            )
                            nc.scalar.activation(
                                so[:, o2 : o2 + v], so2_sl[:, o2 : o2 + v],
                                AF.Sigmoid, bias=bc[:, 6 + half : 7 + half],
                            )
                    else:
                        nc.scalar.activation(v2(tg[:], 2), v2(tg_sl, 2), AF.Tanh)
                        nc.scalar.activation(v2(sf[:], 2), v2(sf_sl, 2), AF.Sigmoid)
                        nc.scalar.activation(v2(so[:], 2), v2(so2_sl, 2), AF.Sigmoid)
                    sig_i, sig_f = si[:], sf[:]
                    sig_o = so[:]

                nc.vector.tensor_tensor(
                    v2(t1[:], 2), v2(sig_i, 2), v2(tg[:], 2), OP.mult
                )
                nc.vector.tensor_tensor(
                    v2(cc[:], 2), v2(sig_f, 2), v2(cc[:], 2), OP.mult
                )
                nc.vector.tensor_tensor(
                    v2(cc[:], 2), v2(cc[:], 2), v2(t1[:], 2), OP.add
                )
                nc.scalar.activation(v2(tcn[:], 2), v2(cc[:], 2), AF.Tanh)
                if L["last"]:
                    hf = opool.tile([128, 512], F32, tag=f"hout_{d}", name=f"hout_{d}")
                    nc.vector.tensor_tensor(hf[:], sig_o, tcn[:], OP.mult)
                    nc.sync.dma_start(out_d[d][:], hf[:])
                elif any_np and L["s"] >= h8_from:
                    # next step's h-projection runs in fp8 DoubleRow
                    nc.vector.tensor_tensor(
                        v2(h8_t[d][:], 2), v2(sig_o, 2), v2(tcn[:], 2), OP.mult
                    )
                else:
                    nc.vector.tensor_tensor(
                        v2(h_t[d][:], 2), v2(sig_o, 2), v2(tcn[:], 2), OP.mult
                    )

            # software pipeline: emit step s's x-projection before step
            # s-1's recurrence/chain whenever their PSUM banks are disjoint
            # (rotation bands), so the in-order PE queue never head-of-line
            # blocks lookahead work behind a stalled h-projection
            mtile = None
            pend = None
            for s in range(t_steps):
                v = int(V[s])
                fp8_step = s >= s0
                xts = {}
                for d in DIRS:
                    if fp8_step:
                        xts[d] = xpool.tile(
                            [128, 2, 2, NS], FP8, tag=f"x8_{d}", name=f"x8_{d}"
                        )
                        nc.sync.dma_start(
                            xts[d][:, :, :, :v], x8_dram[d][s - s0][:, :, :, :v]
                        )
                    else:
                        xts[d] = xpool.tile(
                            [128, KD, NS], BF16, tag=f"x_{d}", name=f"x_{d}"
                        )
                        nc.sync.dma_start(xts[d][:, :, :v], x_dram[d][s][:, :, :v])
                if s % MB == 0:
                    mw = min(MB, t_steps - s) * NS
                    mtile = mpool.tile([128, MB * NS], BF16, tag="m", name="mtile")
                    nc.sync.dma_start(
                        mtile[:, :mw], mask_d[:, s * NS : s * NS + mw]
                    )
                mrhs = mtile[:, (s % MB) * NS : (s % MB) * NS + v]

                L = mk_layout(s, v)
                for d in DIRS:
                    emit_pass1(L, d, xts[d], mrhs, fp8_step)
                    emit_rest(L, d)

    nc.compile()
    return nc


def _get_nc(t_steps, V):
    key = (t_steps, tuple(V))
    if key not in _NC_CACHE:
        _NC_CACHE[key] = _build(t_steps, V)
    return _NC_CACHE[key]


def _prep_weights(W_ih, W_hh, b):
    """lhsT layouts for one direction."""
    import ml_dtypes

    wdt = ml_dtypes.bfloat16
    f8 = ml_dtypes.float8_e4m3
    wih = np.ascontiguousarray(
        W_ih.T.reshape(128, KD, FH).astype(wdt)
    )  # (p, k) <-> dd = KD*p + k
    # fp8 DoubleRow layout: [p, pr, ko, FH] with dd = pr*256 + ko*128 + p
    wih8 = np.ascontiguousarray(
        W_ih.T.reshape(2, 2, 128, FH).transpose(2, 0, 1, 3).astype(f8)
    )
    whh = np.ascontiguousarray(
        W_hh.T.reshape(KH, 128, FH).transpose(1, 0, 2).astype(wdt)
    )  # (p, kk) <-> hrow = 128*kk + p
    whh8 = np.ascontiguousarray(
        W_hh.T.reshape(KH, 128, FH).transpose(1, 0, 2).astype(f8)
    )
    coef = np.zeros(FH, np.float32)
    coef[: 2 * H] = FORCE       # i, f gates
    coef[3 * H :] = FORCE       # o gate
    bm = np.zeros((128, FH), np.float32)
    bm[0] = b.astype(np.float32)
    bm[1] = coef
    bm = np.ascontiguousarray(bm.astype(wdt))
    # ACT bias columns: chunk order [g4,g5,i0,i1,f2,f3,o6,o7]; i slots zero
    # (i gets bias+mask via the bm matmul)
    border = (4, 5, 0, 1, 2, 3, 6, 7)
    b8 = np.stack([b[c * 128 : (c + 1) * 128] for c in border], axis=1)  # [128, 8]
    b8[:, 2:4] = 0.0
    biasc = np.ascontiguousarray(b8.astype(np.float32))
    return wih, whh, bm, wih8, biasc, whh8


def _prep_core(seqs_c, lens_c, t_steps, s0):
    """Per-core device arrays. seqs_c [NS, T, D], lens_c [NS] (sorted desc)."""
    import ml_dtypes

    bf16 = ml_dtypes.bfloat16
    f8 = ml_dtypes.float8_e4m3
    ns = seqs_c.shape[0]
    shift = t_steps - lens_c  # pad steps per sequence
    src_t = np.arange(t_steps)[None, :] - shift[:, None]      # [NS, t]
    valid = src_t >= 0
    gat = seqs_c[np.arange(ns)[:, None], np.clip(src_t, 0, T - 1)]
    xf = np.where(valid[..., None], gat, np.float32(0.0))     # right-aligned
    xb = seqs_c[:, t_steps - 1 :: -1, :]                      # time-reversed

    def to_dev(x_ntd):
        # [NS, t, D] -> bf16 [t, 128, KD, NS] (dd = KD*p + k) for steps < s0,
        # fp8 [t8, 128, 2, 2, NS] (dd = pr*256 + ko*128 + p) for steps >= s0
        xt = x_ntd.transpose(1, 2, 0)                          # [t, D, NS]
        xbf = np.ascontiguousarray(
            xt.astype(bf16).reshape(t_steps, 128, KD, ns))
        t8 = t_steps - s0
        if t8 > 0:
            x8 = np.ascontiguousarray(
                xt[s0:].reshape(t8, 2, 2, 128, ns)
                .transpose(0, 3, 1, 2, 4).astype(f8))
        else:
            x8 = None
        return xbf, x8

    maskinv = (np.arange(t_steps)[:, None] < shift[None, :]).astype(np.float32)
    maskrhs = np.zeros((128, t_steps * ns), np.float32)
    maskrhs[0] = 1.0
    maskrhs[1] = maskinv.reshape(t_steps * ns)
    maskrhs = np.ascontiguousarray(maskrhs.astype(bf16))
    xfb, xf8 = to_dev(xf)
    xbb, xb8 = to_dev(xb)
    return {"xf": xfb, "xb": xbb, "xf8": xf8, "xb8": xb8, "maskrhs": maskrhs}


def _unfold(hT):
    """[128, KH*NS] device tile -> [NS, H] h matrix."""
    h_rows = np.concatenate([hT[:, i * NS : (i + 1) * NS] for i in range(KH)], axis=0)
    return h_rows.T  # [NS, H]


def _run(inputs, trace=False, t_cap=None, **spmd_kwargs):
    import ml_dtypes

    all_embs = np.asarray(inputs["all_embs"], dtype=np.float32)
    lengths = np.asarray(inputs["lengths"]).astype(np.int64)
    starts = np.asarray(inputs["starts"]).astype(np.int64)

    if np.array_equal(starts, np.arange(N, dtype=np.int64) * T):
        seqs = all_embs.reshape(N, T, D)
    else:
        seqs = all_embs[starts[:, None] + np.arange(T)[None, :]]

    # global sort by length desc, deal round-robin to cores
    order = np.argsort(-lengths, kind="stable")
    t_steps = int(lengths.max())
    if t_cap is not None:
        t_steps = min(t_steps, t_cap)
    core_idx = [order[c::NCORES] for c in range(NCORES)]  # [NCORES][NS]

    # baked active widths: V_s = max over cores of #{len >= t_steps - s}
    Ls = np.stack([np.minimum(lengths[ci], t_steps) for ci in core_idx])  # [NC, NS]
    thr = t_steps - np.arange(t_steps)  # [t]
    V = (Ls[:, None, :] >= thr[None, :, None]).sum(-1).max(0)  # [t]
    V = np.maximum(V, 1)

    w = {}
    for d, (wi, wh, bb) in {
        "f": (inputs["W_ih_f"], inputs["W_hh_f"], inputs["b_f"]),
        "b": (inputs["W_ih_b"], inputs["W_hh_b"], inputs["b_b"]),
    }.items():
        w[d] = _prep_weights(
            np.asarray(wi, np.float32), np.asarray(wh, np.float32),
            np.asarray(bb, np.float32),
        )

    s0 = next((s for s in range(t_steps) if int(V[s]) > VSPLIT), t_steps)
    in_maps = []
    for ci in range(NCORES):
        idx = core_idx[ci]
        m = _prep_core(seqs[idx], np.minimum(lengths[idx], t_steps), t_steps, s0)
        im = {
            "xf": m["xf"], "xb": m["xb"], "maskrhs": m["maskrhs"],
            "wihf": w["f"][0], "whhf": w["f"][1], "bmf": w["f"][2],
            "wihb": w["b"][0], "whhb": w["b"][1], "bmb": w["b"][2],
        }
        if m["xf8"] is not None:
            im.update(
                {
                    "x8f": m["xf8"], "x8b": m["xb8"],
                    "wih8f": w["f"][3], "wih8b": w["b"][3],
                }
            )
        if int(V.max()) > PACKLIM:
            im.update(
                {
                    "biascf": w["f"][4], "biascb": w["b"][4],
                    "whh8f": w["f"][5], "whh8b": w["b"][5],
                }
            )
        in_maps.append(im)

    nc = _get_nc(t_steps, V)
    res = None
    for attempt in range(3):
        try:
            res = run_bass_kernel_spmd(
                nc, in_maps, core_ids=list(range(NCORES)), trace=trace,
                **spmd_kwargs
            )
            break
        except Exception:
            # rare transient NRT_EXEC_UNIT_UNRECOVERABLE right after a
            # fresh NEFF load; a plain re-execute has always recovered
            if attempt == 2:
                raise
            import time as _time

            _time.sleep(2.0)

    out = np.empty((N, 2 * H), np.float32)
    for ci in range(NCORES):
        out[core_idx[ci], :H] = _unfold(res.results[ci]["hTf"])
        out[core_idx[ci], H:] = _unfold(res.results[ci]["hTb"])
    return out, res


def kernel(**inputs) -> np.ndarray:
    out, _ = _run(inputs)
    return out

